# revision 17
# baseline (speedup 1.0000x reference)
"""Trainium2 Bass kernel for an 8-layer dense transformer LM (BigramLanguageModel).

Sharding: 8 NeuronCores = 4 batches x 2 sequence halves (512 tokens/core).
Weights replicated (streamed from HBM); per-layer pair-AllGather of K/V
(replica groups [[0,1],[2,3],[4,5],[6,7]]).

Layout: the residual stream lives TRANSPOSED in SBUF as x^T [D, tok] so that
every matmul (qkv, attention, proj, mlp, lm_head) maps directly onto the
PE's out = lhsT.T @ rhs contract with zero on-device transposes:
  q^T/k^T   = W.T @ h^T           (lhsT = W  [D, H*HD])
  v         = h @ Wv              (lhsT = h^T token-slices)
  s^T       = K @ q^T             (lhsT = k^T head-slice)   [ctx, tok]
  o^T (+rs) = [V | 1].T @ wei^T   (lhsT = v-augmented)
  proj/mlp  = W.T @ act^T         (lhsT = W)
  logits    = x @ W_lm            (lhsT = x^T token-slices)
LayerNorm over D (the partition dim) uses ones-matmul reductions and
K=1 broadcast matmuls. Softmax is computed without max-shift (scores are
O(0.3) by construction) with a multiplicative 0/1 causal mask after exp;
row sums come free via the augmented ones-column of V.
"""

import numpy as np
import ml_dtypes

import concourse.bass as bass
import concourse.bacc as bacc
import concourse.mybir as mybir
import concourse.tile as tile
from concourse.bass_utils import run_bass_kernel_spmd

F32 = mybir.dt.float32
F32R = mybir.dt.float32r
BF16 = mybir.dt.bfloat16
AF = mybir.ActivationFunctionType
ALU = mybir.AluOpType
bf = ml_dtypes.bfloat16

VOCAB = 32000
D = 1024
H = 16
HD = 64
L = 8
T = 1024
B = 4
NTOK = 512          # tokens per core
NCORES = 8
FF = 4 * D
EPS = 1e-5
SCALE = D ** -0.5
DT8 = D // 128      # 8 dim tiles
TI4 = NTOK // 128   # 4 token tiles
HT32 = FF // 128    # 32 hidden tiles
NVC = (VOCAB + 511) // 512  # 63 vocab chunks (62x512 + 256)

# matmul operand dtypes (f32r = fp32 storage, ~2e-4 matmul precision, full speed
# at N>=256; bf16 = ~3e-3). LN/residual arithmetic is f32/f32r throughout.
QKV_DT = BF16
ATT_DT = BF16
WO_DT = BF16
W1_DT = BF16
W2_DT = BF16
LM_DT = BF16

# per-partition param column offsets in the packed pp tensor
PP_LN1G, PP_LN1B, PP_LN2G, PP_LN2B = 0, 64, 128, 192
PP_BO, PP_B2 = 256, 320
PP_B1 = 384          # 8 layers x 32 cols
PP_LNFG, PP_LNFB = 640, 648
PP_COLS = 656


def _np_dt(mdt):
    return {F32: np.float32, F32R: np.float32, BF16: bf}[mdt]


def _build(has_blm: bool):
    nc = bacc.Bacc("TRN2", target_bir_lowering=False, debug=False,
                   num_devices=NCORES)

    def din(name, shape, dt):
        return nc.dram_tensor(name, shape, dt, kind="ExternalInput").ap()

    wq = din("wq", [L, D, D], QKV_DT)
    wk = din("wk", [L, D, D], QKV_DT)
    wv = din("wv", [L, D, D], QKV_DT)
    wo = din("wo", [L, D, D], WO_DT)
    w1 = din("w1", [L, D, FF], W1_DT)
    w2 = din("w2", [L, FF, D], W2_DT)
    pp = din("pp", [128, PP_COLS], F32)
    wlm = din("wlm", [D, VOCAB], LM_DT)
    blm = din("blm", [1, VOCAB], LM_DT)
    idm = din("idm", [128, 128], F32)
    x0tok = din("x0tok", [D, NTOK], F32R)
    x0pos = din("x0pos", [D, NTOK], F32R)
    maskd = din("maskd", [128, 8 * NTOK], ATT_DT)
    wg = din("wg", [D, NTOK], LM_DT)
    blmtg = din("blmtg", [1, NTOK], LM_DT)

    logits_o = nc.dram_tensor("logits_o", [NTOK, VOCAB], F32,
                              kind="ExternalOutput").ap()
    loss_o = nc.dram_tensor("loss_o", [1, 1], F32, kind="ExternalOutput").ap()

    with tile.TileContext(nc) as tc:
        _emit(nc, tc, locals(), has_blm)
    nc.compile()
    return nc


def _emit(nc, tc, t_, has_blm):
    wq, wk, wv, wo, w1, w2 = t_["wq"], t_["wk"], t_["wv"], t_["wo"], t_["w1"], t_["w2"]
    pp, wlm, blm, idm = t_["pp"], t_["wlm"], t_["blm"], t_["idm"]
    x0tok, x0pos, maskd, wg = t_["x0tok"], t_["x0pos"], t_["maskd"], t_["wg"]
    blmtg = t_["blmtg"]
    logits_o, loss_o = t_["logits_o"], t_["loss_o"]

    ctx_mgr = []

    def pool(name, bufs, space="SBUF"):
        p = tc.tile_pool(name=name, bufs=bufs, space=space)
        ctx_mgr.append(p)
        return p.__enter__()

    # NOTE: slots are reserved statically per (pool, tag): SBUF usage =
    # sum over tags of bufs * tile_bytes. Tags are deliberately shared
    # between phase-disjoint tensors (kT->q tags, vloc->kctx tags,
    # lm-stream->kv tags) to stay under the 208KB/partition budget.
    const = pool("const", 1)
    xp = pool("xp", 1)           # residual x^T, 8 persistent tiles
    hbp = pool("hbp", 2)         # LN outputs h^T (matmul-operand dtype)
    qp = pool("qp", 2)           # q^T tiles; also k^T staging + lm stream
    kcp = pool("kcp", 1)         # k^T context [128,1024]; also v staging
    vcp = pool("vcp", 1)         # v aug context [128,1040]
    wp = pool("wp", 4)           # weight stream [128,512]
    wvp = pool("wvp", 3)         # wv stream [128,512]
    weip = pool("weip", 3)       # wei^T tiles
    op = pool("op", 1)           # o^T tiles
    up = pool("up", 1)           # u^T tiles (mlp hidden), 32 tags
    lgp = pool("lgp", 3)         # logits eviction
    scr = pool("scr", 3)         # scratch [128,512]
    smp = pool("smp", 4)         # small scratch [1,512]
    rsp = pool("rsp", 1)         # softmax row-sums [1, H*NTOK]
    sep = pool("sep", 1)         # persistent exp-sum accumulators
    wgp = pool("wgp", 4)         # wg gather stream [128,128]
    dram = pool("dram", 2, space="DRAM")

    pg = pool("pg", 4, space="PSUM")     # main GEMM groups + s^T
    po = pool("po", 2, space="PSUM")     # attention o accumulators
    ps = pool("ps", 2, space="PSUM")     # stats / broadcasts / small

    # ---- constants ----
    ones_f = const.tile([128, 1], F32)
    nc.vector.memset(ones_f[:], 1.0)
    ones_r = const.tile([128, 1], F32R)
    nc.vector.tensor_copy(ones_r[:], ones_f[:])
    oinvd_f = const.tile([128, 1], F32)
    nc.vector.memset(oinvd_f[:], 1.0 / D)
    oinvd_r = const.tile([128, 1], F32R)
    nc.vector.tensor_copy(oinvd_r[:], oinvd_f[:])
    ones1_f = const.tile([1, 128], F32)
    nc.vector.memset(ones1_f[:], 1.0)
    ones1_r = const.tile([1, 128], F32R)
    nc.vector.tensor_copy(ones1_r[:], ones1_f[:])
    ones1_b = const.tile([1, 128], BF16)
    nc.vector.memset(ones1_b[:], 1.0)

    e_lo_f = const.tile([1, 128], F32)
    nc.vector.memset(e_lo_f[:, 0:64], 1.0)
    nc.vector.memset(e_lo_f[:, 64:128], 0.0)
    e_lo_r = const.tile([1, 128], F32R)
    nc.vector.tensor_copy(e_lo_r[:], e_lo_f[:])
    e_hi_f = const.tile([1, 128], F32)
    nc.vector.memset(e_hi_f[:, 0:64], 0.0)
    nc.vector.memset(e_hi_f[:, 64:128], 1.0)
    e_hi_r = const.tile([1, 128], F32R)
    nc.vector.tensor_copy(e_hi_r[:], e_hi_f[:])

    ppt = const.tile([128, PP_COLS], F32)
    nc.sync.dma_start(ppt[:], pp[:])
    idt = const.tile([128, 128], F32)
    nc.sync.dma_start(idt[:], idm[:])
    maskt = const.tile([128, 8 * NTOK], ATT_DT)
    nc.sync.dma_start(maskt[:], maskd[:])
    if has_blm:
        blmt = const.tile([1, VOCAB], LM_DT)
        nc.sync.dma_start(blmt[:], blm[:])
    blmtgt = const.tile([1, NTOK], LM_DT)
    nc.sync.dma_start(blmtgt[:], blmtg[:])
    # ---- residual stream init: x^T = tok^T + pos^T ----
    xT = []
    for dt in range(DT8):
        xt = xp.tile([128, NTOK], F32R, tag=f"x{dt}", name=f"xT{dt}")
        nc.sync.dma_start(xt[:], x0tok[dt * 128:(dt + 1) * 128, :])
        sc = scr.tile([128, NTOK], F32R, tag="scr")
        nc.sync.dma_start(sc[:], x0pos[dt * 128:(dt + 1) * 128, :])
        nc.vector.tensor_add(xt[:], xt[:], sc[:])
        xT.append(xt)

    def layer_norm(g_col, b_col, out_dt, out_pool):
        """LN over partition dim of xT -> list of 8 [128,NTOK] tiles."""
        mu_ps = ps.tile([128, NTOK], F32, tag="st")
        for dt in range(DT8):
            nc.tensor.matmul(mu_ps[0:1, :], oinvd_r[:], xT[dt][:],
                             start=(dt == 0), stop=(dt == DT8 - 1))
        e2_ps = ps.tile([128, NTOK], F32, tag="st")
        for dt in range(DT8):
            sq = scr.tile([128, NTOK], F32R, tag="scr")
            nc.vector.tensor_mul(sq[:], xT[dt][:], xT[dt][:])
            nc.tensor.matmul(e2_ps[0:1, :], oinvd_r[:], sq[:],
                             start=(dt == 0), stop=(dt == DT8 - 1))
        mu_sb = smp.tile([1, NTOK], F32R, tag="sm")
        nc.scalar.copy(mu_sb[:], mu_ps[0:1, :])
        mu2 = smp.tile([1, NTOK], F32, tag="sm")
        nc.vector.tensor_mul(mu2[:], mu_sb[:], mu_sb[:])
        veps = smp.tile([1, NTOK], F32, tag="sm")
        nc.vector.scalar_tensor_tensor(veps[:], e2_ps[0:1, :], EPS, mu2[:],
                                       op0=ALU.add, op1=ALU.subtract)
        rv = smp.tile([1, NTOK], F32, tag="sm")
        nc.vector.reciprocal(rv[:], veps[:])
        rstd = smp.tile([1, NTOK], F32R, tag="sm")
        nc.scalar.activation(rstd[:], rv[:], AF.Sqrt)
        mub_ps = ps.tile([128, NTOK], F32, tag="st")
        nc.tensor.matmul(mub_ps[:], ones1_r[:], mu_sb[:], start=True, stop=True)
        rsb_ps = ps.tile([128, NTOK], F32, tag="st")
        nc.tensor.matmul(rsb_ps[:], ones1_r[:], rstd[:], start=True, stop=True)
        out = []
        for dt in range(DT8):
            t1 = scr.tile([128, NTOK], F32R, tag="scr")
            nc.vector.tensor_sub(t1[:], xT[dt][:], mub_ps[:])
            nc.vector.tensor_mul(t1[:], t1[:], rsb_ps[:])
            ht = out_pool.tile([128, NTOK], out_dt, tag=f"h{dt}", name=f"hT{dt}")
            nc.vector.tensor_scalar(ht[:], t1[:],
                                    ppt[:, g_col + dt:g_col + dt + 1],
                                    ppt[:, b_col + dt:b_col + dt + 1],
                                    op0=ALU.mult, op1=ALU.add)
            out.append(ht)
        return out

    def proj_T(w_dram, l, rhs_tiles, n_m, w_dt, evict):
        """out^T [M, tok] = W[:, :].T @ rhs^T; M = n_m*128, contraction over
        len(rhs_tiles)*128. evict(m, psum) consumes each output tile."""
        kt_n = len(rhs_tiles)
        for half0 in range(0, n_m, 4):
            mh = min(4, n_m - half0)
            psums = [pg.tile([128, NTOK], F32, tag="pg", name=f"pgm{m}") for m in range(mh)]
            for kt in range(kt_n):
                wt = wp.tile([128, 4 * 128], w_dt, tag="w")
                nc.sync.dma_start(
                    wt[:, :mh * 128],
                    w_dram[l, kt * 128:(kt + 1) * 128,
                           half0 * 128:(half0 + mh) * 128])
                for m in range(mh):
                    nc.tensor.matmul(psums[m][:],
                                     wt[:, m * 128:(m + 1) * 128],
                                     rhs_tiles[kt][:],
                                     start=(kt == 0), stop=(kt == kt_n - 1))
            for m in range(mh):
                evict(half0 + m, psums[m])

    # kv_in rows: k^T [1024,512] -> rows 0:1024; v 4x[128,1024] -> rows 1024:2048
    kv_in_sh = [4 * NTOK, NTOK]
    kv_out_sh = [8 * NTOK, NTOK]

    for l in range(L):
        # ---- LN1 ----
        hq = layer_norm(PP_LN1G + l * 8, PP_LN1B + l * 8, QKV_DT, hbp)

        # ---- k^T, v, AG; then q^T ----
        kT = []

        def ev_k(m, psum):
            kt_t = qp.tile([128, NTOK], ATT_DT, tag=f"q{m}", name=f"kT{m}")
            nc.scalar.copy(kt_t[:], psum[:])
            kT.append(kt_t)

        proj_T(wk, l, hq, DT8, QKV_DT, ev_k)

        vloc = []
        for nc2 in range(2):
            psums = [pg.tile([128, NTOK], F32, tag="pg", name=f"pgv{ti}") for ti in range(TI4)]
            for dt in range(DT8):
                wt = wvp.tile([128, NTOK], QKV_DT, tag="wv")
                nc.sync.dma_start(
                    wt[:], wv[l, dt * 128:(dt + 1) * 128,
                              nc2 * 512:(nc2 + 1) * 512])
                for ti in range(TI4):
                    nc.tensor.matmul(psums[ti][:],
                                     hq[dt][:, ti * 128:(ti + 1) * 128],
                                     wt[:],
                                     start=(dt == 0), stop=(dt == DT8 - 1))
            for ti in range(TI4):
                if nc2 == 0:
                    vloc.append(kcp.tile([128, 2 * NTOK], ATT_DT, tag=f"kc{ti}", name=f"vloc{ti}"))
                nc.scalar.copy(vloc[ti][:, nc2 * 512:(nc2 + 1) * 512], psums[ti][:])

        kv_in = dram.tile(kv_in_sh, ATT_DT, tag="kvi")
        for dt in range(DT8):
            nc.sync.dma_start(kv_in[dt * 128:(dt + 1) * 128, :], kT[dt][:])
        for ti in range(TI4):
            nc.sync.dma_start(
                kv_in[2 * NTOK + ti * 256:2 * NTOK + (ti + 1) * 256, :],
                vloc[ti][:])
        kv_out = dram.tile(kv_out_sh, ATT_DT, tag="kvo")
        nc.gpsimd.collective_compute(
            "AllGather", ALU.bypass,
            replica_groups=[[0, 1], [2, 3], [4, 5], [6, 7]],
            ins=[kv_in.opt()], outs=[kv_out.opt()])

        qT = []

        def ev_q(m, psum):
            q_t = qp.tile([128, NTOK], ATT_DT, tag=f"q{m}", name=f"qT{m}")
            nc.scalar.copy(q_t[:], psum[:])
            qT.append(q_t)

        proj_T(wq, l, hq, DT8, QKV_DT, ev_q)

        # ---- context assembly from AG output ----
        kctx = []
        for dt in range(DT8):
            kc = kcp.tile([128, T], ATT_DT, tag=f"kc{dt}", name=f"kctx{dt}")
            nc.sync.dma_start(kc[:, 0:NTOK], kv_out[dt * 128:(dt + 1) * 128, :])
            nc.sync.dma_start(kc[:, NTOK:T],
                              kv_out[4 * NTOK + dt * 128:4 * NTOK + (dt + 1) * 128, :])
            kctx.append(kc)
        vaug = []
        for ct in range(8):
            va = vcp.tile([128, H * (HD + 1)], ATT_DT, tag=f"vc{ct}", name=f"vaug{ct}")
            if ct < 4:
                src = kv_out[2 * NTOK + ct * 256:2 * NTOK + (ct + 1) * 256, :]
            else:
                src = kv_out[6 * NTOK + (ct - 4) * 256:6 * NTOK + (ct - 3) * 256, :]
            nc.sync.dma_start(
                va[:].rearrange("p (h e) -> p h e", h=H)[:, :, 0:HD],
                src.rearrange("(p r) n -> p (r n)", r=2)
                   .rearrange("p (h e) -> p h e", h=H))
            nc.vector.memset(
                va[:].rearrange("p (h e) -> p h e", h=H)[:, :, HD:HD + 1], 1.0)
            vaug.append(va)

        # ---- attention ----
        rs = rsp.tile([1, H * NTOK], F32R, tag="rs")
        oT = [op.tile([128, NTOK], WO_DT, tag=f"o{dt}", name=f"oT{dt}") for dt in range(DT8)]
        for h in range(H):
            hoff = (h % 2) * 64
            ops = po.tile([HD + 1, NTOK], F32, tag="po")
            for ct in range(8):
                c0 = 0 if ct < 4 else (ct - 4) * 128
                sps = pg.tile([128, NTOK], F32, tag="pg")
                nc.tensor.matmul(sps[:, c0:NTOK],
                                 kctx[h // 2][hoff:hoff + 64,
                                              ct * 128:(ct + 1) * 128],
                                 qT[h // 2][hoff:hoff + 64, c0:NTOK],
                                 start=True, stop=True)
                wei = weip.tile([128, NTOK], ATT_DT, tag="wei")
                nc.scalar.activation(wei[:, c0:NTOK], sps[:, c0:NTOK],
                                     AF.Exp, scale=SCALE)
                nc.vector.tensor_mul(wei[:, c0:NTOK], wei[:, c0:NTOK],
                                     maskt[:, ct * NTOK + c0:(ct + 1) * NTOK])
                nc.tensor.matmul(ops[:, c0:NTOK],
                                 vaug[ct][:, h * 65:h * 65 + 65],
                                 wei[:, c0:NTOK],
                                 start=(ct == 0), stop=(ct == 7))
            nc.scalar.copy(oT[h // 2][hoff:hoff + 64, :], ops[0:HD, :])
            nc.scalar.copy(rs[0:1, h * NTOK:(h + 1) * NTOK], ops[HD:HD + 1, :])

        for dt in range(DT8):
            bps = ps.tile([128, NTOK], F32, tag="st")
            nc.tensor.matmul(bps[:], e_lo_r[:],
                             rs[0:1, (2 * dt) * NTOK:(2 * dt + 1) * NTOK],
                             start=True, stop=False)
            nc.tensor.matmul(bps[:], e_hi_r[:],
                             rs[0:1, (2 * dt + 1) * NTOK:(2 * dt + 2) * NTOK],
                             start=False, stop=True)
            rcp = scr.tile([128, NTOK], ATT_DT, tag="rcp")
            with nc.allow_low_precision(reason="softmax denom recip"):
                nc.vector.reciprocal(rcp[:], bps[:])
            nc.vector.tensor_mul(oT[dt][:], oT[dt][:], rcp[:])

        # ---- attn out proj + residual ----
        def ev_wo(m, psum):
            nc.vector.scalar_tensor_tensor(
                xT[m][:], psum[:], ppt[:, PP_BO + l * 8 + m:PP_BO + l * 8 + m + 1],
                xT[m][:], op0=ALU.add, op1=ALU.add)

        proj_T(wo, l, oT, DT8, WO_DT, ev_wo)

        # ---- LN2 + MLP ----
        h2 = layer_norm(PP_LN2G + l * 8, PP_LN2B + l * 8, W1_DT, hbp)

        uT = []

        def ev_u(m, psum):
            u_t = up.tile([128, NTOK], W2_DT, tag=f"u{m}", name=f"uT{m}")
            nc.scalar.activation(u_t[:], psum[:], AF.Relu,
                                 bias=ppt[:, PP_B1 + l * 32 + m:PP_B1 + l * 32 + m + 1])
            uT.append(u_t)

        proj_T(w1, l, h2, HT32, W1_DT, ev_u)

        def ev_w2(m, psum):
            nc.vector.scalar_tensor_tensor(
                xT[m][:], psum[:], ppt[:, PP_B2 + l * 8 + m:PP_B2 + l * 8 + m + 1],
                xT[m][:], op0=ALU.add, op1=ALU.add)

        proj_T(w2, l, uT, DT8, W2_DT, ev_w2)

    # ---- final LN + lm head + loss ----
    xb = layer_norm(PP_LNFG, PP_LNFB, LM_DT, hbp)

    sums = [sep.tile([128, NVC + 1], F32, tag=f"se{ti}", name=f"sums{ti}") for ti in range(TI4)]
    for vc in range(NVC):
        nv = min(512, VOCAB - vc * 512)
        wts = []
        for dt in range(DT8):
            wt = qp.tile([128, NTOK], LM_DT, tag=f"q{dt}", name=f"wlmt{dt}")
            nc.sync.dma_start(wt[:, :nv],
                              wlm[dt * 128:(dt + 1) * 128, vc * 512:vc * 512 + nv])
            wts.append(wt)
        for ti in range(TI4):
            pps = pg.tile([128, NTOK], F32, tag="pg")
            if has_blm:
                nc.tensor.matmul(pps[:, :nv], ones1_b[:],
                                 blmt[:, vc * 512:vc * 512 + nv],
                                 start=True, stop=False)
            for dt in range(DT8):
                nc.tensor.matmul(pps[:, :nv],
                                 xb[dt][:, ti * 128:(ti + 1) * 128],
                                 wts[dt][:, :nv],
                                 start=(dt == 0 and not has_blm),
                                 stop=(dt == DT8 - 1))
            lg = lgp.tile([128, NTOK], F32, tag="lg")
            nc.scalar.copy(lg[:, :nv], pps[:, :nv])
            nc.sync.dma_start(
                logits_o[ti * 128:(ti + 1) * 128, vc * 512:vc * 512 + nv],
                lg[:, :nv])
            esc = scr.tile([128, NTOK], F32R, tag="scr")
            nc.scalar.activation(esc[:, :nv], pps[:, :nv], AF.Exp,
                                 accum_out=sums[ti][:, vc:vc + 1])

    lps = ps.tile([128, NTOK], F32, tag="st")
    for ti in range(TI4):
        s_t = smp.tile([128, 1], F32, tag="s1")
        nc.vector.reduce_sum(s_t[:], sums[ti][:, 0:NVC], axis=mybir.AxisListType.X)
        lse = smp.tile([128, 1], F32, tag="s1")
        nc.scalar.activation(lse[:], s_t[:], AF.Ln)

        tps = ps.tile([128, NTOK], F32, tag="st")
        if has_blm:
            nc.tensor.matmul(tps[:, 0:128], ones1_b[:],
                             blmtgt[:, ti * 128:(ti + 1) * 128],
                             start=True, stop=False)
        for dt in range(DT8):
            wgt_t = wgp.tile([128, 128], LM_DT, tag="wg", name=f"wgt{ti}_{dt}")
            nc.sync.dma_start(wgt_t[:],
                              wg[dt * 128:(dt + 1) * 128, ti * 128:(ti + 1) * 128])
            nc.tensor.matmul(tps[:, 0:128],
                             xb[dt][:, ti * 128:(ti + 1) * 128],
                             wgt_t[:],
                             start=(dt == 0 and not has_blm),
                             stop=(dt == DT8 - 1))
        dsc = scr.tile([128, NTOK], F32, tag="scr")
        nc.vector.tensor_mul(dsc[:, 0:128], tps[:, 0:128], idt[:])
        tg = smp.tile([128, 1], F32, tag="s1")
        nc.vector.reduce_sum(tg[:], dsc[:, 0:128], axis=mybir.AxisListType.X)
        lv = smp.tile([128, 1], F32, tag="s1")
        nc.vector.tensor_sub(lv[:], lse[:], tg[:])
        nc.tensor.matmul(lps[0:1, 0:1], lv[:], ones_f[:],
                         start=(ti == 0), stop=(ti == TI4 - 1))
    lsb = smp.tile([1, 1], F32, tag="s1")
    nc.scalar.copy(lsb[:], lps[0:1, 0:1])
    nc.sync.dma_start(loss_o[:], lsb[:])

    for p in reversed(ctx_mgr):
        p.__exit__(None, None, None)


_CACHE = {}


def _get_nc(has_blm):
    key = ("nc", has_blm)
    if key not in _CACHE:
        _CACHE[key] = _build(has_blm)
    return _CACHE[key]


def _pack_pp(vec, lrows=True):
    """[L, n] (or [n]) f32 -> [128, n*L/128] with col (l, c) = vec[l, c*128:...]."""
    v = np.asarray(vec, np.float32)
    if v.ndim == 1:
        v = v[None]
    Lx, n = v.shape
    return v.reshape(Lx, n // 128, 128).transpose(2, 0, 1).reshape(128, -1)


def kernel(idx, targets, params):
    idx = np.asarray(idx)
    targets = np.asarray(targets)
    p = {k: np.asarray(v) for k, v in params.items()
         if not isinstance(v, dict)}
    lay = {k: np.asarray(v) for k, v in params["layers"].items()}

    qkv_np = _np_dt(QKV_DT)
    att_np = _np_dt(ATT_DT)
    wo_np = _np_dt(WO_DT)
    w1_np = _np_dt(W1_DT)
    w2_np = _np_dt(W2_DT)
    lm_np = _np_dt(LM_DT)

    b_lm = p["b_lm"].astype(np.float32)
    has_blm = bool(np.any(b_lm != 0))

    shared = {
        "wq": np.ascontiguousarray(
            lay["wq"].transpose(0, 2, 1, 3).reshape(L, D, D).astype(qkv_np)),
        "wk": np.ascontiguousarray(
            lay["wk"].transpose(0, 2, 1, 3).reshape(L, D, D).astype(qkv_np)),
        "wv": np.ascontiguousarray(
            lay["wv"].transpose(0, 2, 1, 3).reshape(L, D, D).astype(qkv_np)),
        "wo": np.ascontiguousarray(lay["wo"].astype(wo_np)),
        "w1": np.ascontiguousarray(lay["w1"].astype(w1_np)),
        "w2": np.ascontiguousarray(lay["w2"].astype(w2_np)),
        "wlm": np.ascontiguousarray(p["w_lm"].astype(lm_np)),
        "blm": np.ascontiguousarray(b_lm[None, :].astype(lm_np)),
        "idm": np.eye(128, dtype=np.float32),
    }
    ppm = np.zeros((128, PP_COLS), np.float32)
    ppm[:, PP_LN1G:PP_LN1G + 64] = _pack_pp(lay["ln1_g"])
    ppm[:, PP_LN1B:PP_LN1B + 64] = _pack_pp(lay["ln1_b"])
    ppm[:, PP_LN2G:PP_LN2G + 64] = _pack_pp(lay["ln2_g"])
    ppm[:, PP_LN2B:PP_LN2B + 64] = _pack_pp(lay["ln2_b"])
    ppm[:, PP_BO:PP_BO + 64] = _pack_pp(lay["bo"])
    ppm[:, PP_B2:PP_B2 + 64] = _pack_pp(lay["b2"])
    ppm[:, PP_B1:PP_B1 + 256] = _pack_pp(lay["b1"])
    ppm[:, PP_LNFG:PP_LNFG + 8] = _pack_pp(p["lnf_g"])
    ppm[:, PP_LNFB:PP_LNFB + 8] = _pack_pp(p["lnf_b"])
    shared["pp"] = ppm

    tok_emb = p["tok_emb"].astype(np.float32)
    pos_emb = p["pos_emb"].astype(np.float32)
    w_lm_f = p["w_lm"].astype(np.float32)

    in_maps = []
    for c in range(NCORES):
        bidx, half = c // 2, c % 2
        sl = slice(half * NTOK, (half + 1) * NTOK)
        toks = idx[bidx, sl]
        tgts = targets[bidx, sl]
        m = dict(shared)
        m["x0tok"] = np.ascontiguousarray(tok_emb[toks].T)
        m["x0pos"] = np.ascontiguousarray(pos_emb[sl].T)
        qpos = half * NTOK + np.arange(NTOK)
        mask = np.zeros((128, 8 * NTOK), np.float32)
        for ct in range(8):
            kpos = ct * 128 + np.arange(128)
            mask[:, ct * NTOK:(ct + 1) * NTOK] = (
                kpos[:, None] <= qpos[None, :]).astype(np.float32)
        m["maskd"] = mask.astype(att_np)
        m["wg"] = np.ascontiguousarray(w_lm_f[:, tgts].astype(lm_np))
        m["blmtg"] = np.ascontiguousarray(b_lm[tgts][None, :].astype(lm_np))
        in_maps.append(m)

    nc = _get_nc(has_blm)
    res = run_bass_kernel_spmd(nc, in_maps, core_ids=list(range(NCORES)),
                               **_RUN_KW)

    logits = np.empty((B, T, VOCAB), np.float32)
    loss_sum = 0.0
    for c in range(NCORES):
        bidx, half = c // 2, c % 2
        logits[bidx, half * NTOK:(half + 1) * NTOK, :] = res.results[c]["logits_o"]
        loss_sum += float(res.results[c]["loss_o"][0, 0])
    loss = np.float32(loss_sum / (B * T))
    _CACHE["last_results"] = res
    return logits, loss


# knobs for test.py: extra kwargs for run_bass_kernel_spmd (e.g. trace=True),
# and _CACHE["last_results"] holds the BassKernelResults of the last run.
_RUN_KW = {}


# revision 18
# speedup vs baseline: 445.5307x; 445.5307x over previous
"""Trainium2 Bass kernel for an 8-layer dense transformer LM (BigramLanguageModel).

Sharding: 8 NeuronCores = 4 batches x 2 sequence halves (512 tokens/core).
Weights replicated (streamed from HBM); per-layer pair-AllGather of K/V
(replica groups [[0,1],[2,3],[4,5],[6,7]]).

Layout: the residual stream lives TRANSPOSED in SBUF as x^T [D, tok] so that
every matmul (qkv, attention, proj, mlp, lm_head) maps directly onto the
PE's out = lhsT.T @ rhs contract with zero on-device transposes:
  q^T/k^T   = W.T @ h^T           (lhsT = W  [D, H*HD])
  v         = h @ Wv              (lhsT = h^T token-slices)
  s^T       = K @ q^T             (lhsT = k^T head-slice)   [ctx, tok]
  o^T (+rs) = [V | 1].T @ wei^T   (lhsT = v-augmented)
  proj/mlp  = W.T @ act^T         (lhsT = W)
  logits    = x @ W_lm            (lhsT = x^T token-slices)
LayerNorm over D (the partition dim) uses ones-matmul reductions and
K=1 broadcast matmuls. Softmax is computed without max-shift (scores are
O(0.3) by construction) with a multiplicative 0/1 causal mask after exp;
row sums come free via the augmented ones-column of V.
"""

import numpy as np
import ml_dtypes

import concourse.bass as bass
import concourse.bacc as bacc
import concourse.mybir as mybir
import concourse.tile as tile
from concourse.bass_utils import run_bass_kernel_spmd

F32 = mybir.dt.float32
F32R = mybir.dt.float32r
BF16 = mybir.dt.bfloat16
AF = mybir.ActivationFunctionType
ALU = mybir.AluOpType
bf = ml_dtypes.bfloat16

VOCAB = 32000
D = 1024
H = 16
HD = 64
L = 8
T = 1024
B = 4
NTOK = 512          # tokens per core
NCORES = 8
FF = 4 * D
EPS = 1e-5
SCALE = D ** -0.5
DT8 = D // 128      # 8 dim tiles
TI4 = NTOK // 128   # 4 token tiles
HT32 = FF // 128    # 32 hidden tiles
NVC = (VOCAB + 511) // 512  # 63 vocab chunks (62x512 + 256)

# matmul operand dtypes (f32r = fp32 storage, ~2e-4 matmul precision, full speed
# at N>=256; bf16 = ~3e-3). LN/residual arithmetic is f32/f32r throughout.
QKV_DT = BF16
ATT_DT = BF16
WO_DT = BF16
W1_DT = BF16
W2_DT = BF16
LM_DT = BF16

# per-partition param column offsets in the packed pp tensor
PP_LN1G, PP_LN1B, PP_LN2G, PP_LN2B = 0, 64, 128, 192
PP_BO, PP_B2 = 256, 320
PP_B1 = 384          # 8 layers x 32 cols
PP_LNFG, PP_LNFB = 640, 648
PP_COLS = 656


def _np_dt(mdt):
    return {F32: np.float32, F32R: np.float32, BF16: bf}[mdt]


def _build(has_blm: bool):
    nc = bacc.Bacc("TRN2", target_bir_lowering=False, debug=False,
                   num_devices=NCORES)

    def din(name, shape, dt):
        return nc.dram_tensor(name, shape, dt, kind="ExternalInput").ap()

    wq = din("wq", [L, D, D], QKV_DT)
    wk = din("wk", [L, D, D], QKV_DT)
    wv = din("wv", [L, D, D], QKV_DT)
    wo = din("wo", [L, D, D], WO_DT)
    w1 = din("w1", [L, D, FF], W1_DT)
    w2 = din("w2", [L, FF, D], W2_DT)
    pp = din("pp", [128, PP_COLS], F32)
    wlm = din("wlm", [D, VOCAB], LM_DT)
    blm = din("blm", [1, VOCAB], LM_DT)
    idm = din("idm", [128, 128], F32)
    x0tok = din("x0tok", [D, NTOK], F32R)
    x0pos = din("x0pos", [D, NTOK], F32R)
    maskd = din("maskd", [128, 8 * NTOK], ATT_DT)
    wg = din("wg", [D, NTOK], LM_DT)
    blmtg = din("blmtg", [1, NTOK], LM_DT)

    logits_o = nc.dram_tensor("logits_o", [NTOK, VOCAB], F32,
                              kind="ExternalOutput").ap()
    loss_o = nc.dram_tensor("loss_o", [1, 1], F32, kind="ExternalOutput").ap()

    with tile.TileContext(nc) as tc:
        _emit(nc, tc, locals(), has_blm)
    nc.compile()
    return nc


def _emit(nc, tc, t_, has_blm):
    wq, wk, wv, wo, w1, w2 = t_["wq"], t_["wk"], t_["wv"], t_["wo"], t_["w1"], t_["w2"]
    pp, wlm, blm, idm = t_["pp"], t_["wlm"], t_["blm"], t_["idm"]
    x0tok, x0pos, maskd, wg = t_["x0tok"], t_["x0pos"], t_["maskd"], t_["wg"]
    blmtg = t_["blmtg"]
    logits_o, loss_o = t_["logits_o"], t_["loss_o"]

    ctx_mgr = []

    def pool(name, bufs, space="SBUF"):
        p = tc.tile_pool(name=name, bufs=bufs, space=space)
        ctx_mgr.append(p)
        return p.__enter__()

    # NOTE: slots are reserved statically per (pool, tag): SBUF usage =
    # sum over tags of bufs * tile_bytes. Tags are deliberately shared
    # between phase-disjoint tensors (kT->q tags, vloc->kctx tags,
    # lm-stream->kv tags) to stay under the 208KB/partition budget.
    const = pool("const", 1)
    xp = pool("xp", 1)           # residual x^T, 8 persistent tiles
    hbp = pool("hbp", 2)         # LN outputs h^T (matmul-operand dtype)
    qp = pool("qp", 2)           # q^T tiles; also k^T staging + lm stream
    kcp = pool("kcp", 1)         # k^T context [128,1024]; also v staging
    vcp = pool("vcp", 1)         # v aug context [128,1040]
    wp = pool("wp", 4)           # weight stream [128,512]
    wvp = pool("wvp", 3)         # wv stream [128,512]
    weip = pool("weip", 3)       # wei^T tiles
    op = pool("op", 1)           # o^T tiles
    up = pool("up", 1)           # u^T tiles (mlp hidden), 32 tags
    lgp = pool("lgp", 3)         # logits eviction
    scr = pool("scr", 3)         # scratch [128,512]
    smp = pool("smp", 4)         # small scratch [1,512]
    rsp = pool("rsp", 1)         # softmax row-sums [1, H*NTOK]
    sep = pool("sep", 1)         # persistent exp-sum accumulators
    wgp = pool("wgp", 4)         # wg gather stream [128,128]
    dram = pool("dram", 2, space="DRAM")

    pg = pool("pg", 4, space="PSUM")     # main GEMM groups + s^T
    po = pool("po", 2, space="PSUM")     # attention o accumulators
    ps = pool("ps", 2, space="PSUM")     # stats / broadcasts / small

    # ---- constants ----
    ones_f = const.tile([128, 1], F32)
    nc.vector.memset(ones_f[:], 1.0)
    ones_r = const.tile([128, 1], F32R)
    nc.vector.tensor_copy(ones_r[:], ones_f[:])
    oinvd_f = const.tile([128, 1], F32)
    nc.vector.memset(oinvd_f[:], 1.0 / D)
    oinvd_r = const.tile([128, 1], F32R)
    nc.vector.tensor_copy(oinvd_r[:], oinvd_f[:])
    ones1_f = const.tile([1, 128], F32)
    nc.vector.memset(ones1_f[:], 1.0)
    ones1_r = const.tile([1, 128], F32R)
    nc.vector.tensor_copy(ones1_r[:], ones1_f[:])
    ones1_b = const.tile([1, 128], BF16)
    nc.vector.memset(ones1_b[:], 1.0)

    e_lo_f = const.tile([1, 128], F32)
    nc.vector.memset(e_lo_f[:, 0:64], 1.0)
    nc.vector.memset(e_lo_f[:, 64:128], 0.0)
    e_lo_r = const.tile([1, 128], F32R)
    nc.vector.tensor_copy(e_lo_r[:], e_lo_f[:])
    e_hi_f = const.tile([1, 128], F32)
    nc.vector.memset(e_hi_f[:, 0:64], 0.0)
    nc.vector.memset(e_hi_f[:, 64:128], 1.0)
    e_hi_r = const.tile([1, 128], F32R)
    nc.vector.tensor_copy(e_hi_r[:], e_hi_f[:])

    ppt = const.tile([128, PP_COLS], F32)
    nc.sync.dma_start(ppt[:], pp[:])
    idt = const.tile([128, 128], F32)
    nc.sync.dma_start(idt[:], idm[:])
    maskt = const.tile([128, 8 * NTOK], ATT_DT)
    nc.sync.dma_start(maskt[:], maskd[:])
    if has_blm:
        blmt = const.tile([1, VOCAB], LM_DT)
        nc.sync.dma_start(blmt[:], blm[:])
    blmtgt = const.tile([1, NTOK], LM_DT)
    nc.sync.dma_start(blmtgt[:], blmtg[:])
    # ---- residual stream init: x^T = tok^T + pos^T ----
    xT = []
    for dt in range(DT8):
        xt = xp.tile([128, NTOK], F32R, tag=f"x{dt}", name=f"xT{dt}")
        nc.sync.dma_start(xt[:], x0tok[dt * 128:(dt + 1) * 128, :])
        sc = scr.tile([128, NTOK], F32R, tag="scr")
        nc.sync.dma_start(sc[:], x0pos[dt * 128:(dt + 1) * 128, :])
        nc.vector.tensor_add(xt[:], xt[:], sc[:])
        xT.append(xt)

    def layer_norm(g_col, b_col, out_dt, out_pool):
        """LN over partition dim of xT -> list of 8 [128,NTOK] tiles."""
        mu_ps = ps.tile([128, NTOK], F32, tag="st")
        for dt in range(DT8):
            nc.tensor.matmul(mu_ps[0:1, :], oinvd_r[:], xT[dt][:],
                             start=(dt == 0), stop=(dt == DT8 - 1))
        e2_ps = ps.tile([128, NTOK], F32, tag="st")
        for dt in range(DT8):
            sq = scr.tile([128, NTOK], F32R, tag="scr")
            nc.vector.tensor_mul(sq[:], xT[dt][:], xT[dt][:])
            nc.tensor.matmul(e2_ps[0:1, :], oinvd_r[:], sq[:],
                             start=(dt == 0), stop=(dt == DT8 - 1))
        mu_sb = smp.tile([1, NTOK], F32R, tag="sm")
        nc.scalar.copy(mu_sb[:], mu_ps[0:1, :])
        mu2 = smp.tile([1, NTOK], F32, tag="sm")
        nc.vector.tensor_mul(mu2[:], mu_sb[:], mu_sb[:])
        veps = smp.tile([1, NTOK], F32, tag="sm")
        nc.vector.scalar_tensor_tensor(veps[:], e2_ps[0:1, :], EPS, mu2[:],
                                       op0=ALU.add, op1=ALU.subtract)
        rv = smp.tile([1, NTOK], F32, tag="sm")
        nc.vector.reciprocal(rv[:], veps[:])
        rstd = smp.tile([1, NTOK], F32R, tag="sm")
        nc.scalar.activation(rstd[:], rv[:], AF.Sqrt)
        mub_ps = ps.tile([128, NTOK], F32, tag="st")
        nc.tensor.matmul(mub_ps[:], ones1_r[:], mu_sb[:], start=True, stop=True)
        rsb_ps = ps.tile([128, NTOK], F32, tag="st")
        nc.tensor.matmul(rsb_ps[:], ones1_r[:], rstd[:], start=True, stop=True)
        out = []
        for dt in range(DT8):
            t1 = scr.tile([128, NTOK], F32R, tag="scr")
            nc.vector.tensor_sub(t1[:], xT[dt][:], mub_ps[:])
            nc.vector.tensor_mul(t1[:], t1[:], rsb_ps[:])
            ht = out_pool.tile([128, NTOK], out_dt, tag=f"h{dt}", name=f"hT{dt}")
            nc.vector.tensor_scalar(ht[:], t1[:],
                                    ppt[:, g_col + dt:g_col + dt + 1],
                                    ppt[:, b_col + dt:b_col + dt + 1],
                                    op0=ALU.mult, op1=ALU.add)
            out.append(ht)
        return out

    def proj_T(w_dram, l, rhs_tiles, n_m, w_dt, evict):
        """out^T [M, tok] = W[:, :].T @ rhs^T; M = n_m*128, contraction over
        len(rhs_tiles)*128. evict(m, psum) consumes each output tile."""
        kt_n = len(rhs_tiles)
        for half0 in range(0, n_m, 4):
            mh = min(4, n_m - half0)
            psums = [pg.tile([128, NTOK], F32, tag="pg", name=f"pgm{m}") for m in range(mh)]
            for kt in range(kt_n):
                wt = wp.tile([128, 4 * 128], w_dt, tag="w")
                nc.sync.dma_start(
                    wt[:, :mh * 128],
                    w_dram[l, kt * 128:(kt + 1) * 128,
                           half0 * 128:(half0 + mh) * 128])
                for m in range(mh):
                    nc.tensor.matmul(psums[m][:],
                                     wt[:, m * 128:(m + 1) * 128],
                                     rhs_tiles[kt][:],
                                     start=(kt == 0), stop=(kt == kt_n - 1))
            for m in range(mh):
                evict(half0 + m, psums[m])

    # kv_in rows: k^T [1024,512] -> rows 0:1024; v 4x[128,1024] -> rows 1024:2048
    kv_in_sh = [4 * NTOK, NTOK]
    kv_out_sh = [8 * NTOK, NTOK]

    for l in range(L):
        # ---- LN1 ----
        hq = layer_norm(PP_LN1G + l * 8, PP_LN1B + l * 8, QKV_DT, hbp)

        # ---- k^T, v, AG; then q^T ----
        kT = []

        def ev_k(m, psum):
            kt_t = qp.tile([128, NTOK], ATT_DT, tag=f"q{m}", name=f"kT{m}")
            nc.scalar.copy(kt_t[:], psum[:])
            kT.append(kt_t)

        proj_T(wk, l, hq, DT8, QKV_DT, ev_k)

        vloc = []
        for nc2 in range(2):
            psums = [pg.tile([128, NTOK], F32, tag="pg", name=f"pgv{ti}") for ti in range(TI4)]
            for dt in range(DT8):
                wt = wvp.tile([128, NTOK], QKV_DT, tag="wv")
                nc.sync.dma_start(
                    wt[:], wv[l, dt * 128:(dt + 1) * 128,
                              nc2 * 512:(nc2 + 1) * 512])
                for ti in range(TI4):
                    nc.tensor.matmul(psums[ti][:],
                                     hq[dt][:, ti * 128:(ti + 1) * 128],
                                     wt[:],
                                     start=(dt == 0), stop=(dt == DT8 - 1))
            for ti in range(TI4):
                if nc2 == 0:
                    vloc.append(kcp.tile([128, 2 * NTOK], ATT_DT, tag=f"kc{ti}", name=f"vloc{ti}"))
                nc.scalar.copy(vloc[ti][:, nc2 * 512:(nc2 + 1) * 512], psums[ti][:])

        kv_in = dram.tile(kv_in_sh, ATT_DT, tag="kvi")
        for dt in range(DT8):
            nc.sync.dma_start(kv_in[dt * 128:(dt + 1) * 128, :], kT[dt][:])
        for ti in range(TI4):
            nc.sync.dma_start(
                kv_in[2 * NTOK + ti * 256:2 * NTOK + (ti + 1) * 256, :],
                vloc[ti][:])
        kv_out = dram.tile(kv_out_sh, ATT_DT, tag="kvo")
        nc.gpsimd.collective_compute(
            "AllGather", ALU.bypass,
            replica_groups=[[0, 1], [2, 3], [4, 5], [6, 7]],
            ins=[kv_in.opt()], outs=[kv_out.opt()])

        qT = []

        def ev_q(m, psum):
            q_t = qp.tile([128, NTOK], ATT_DT, tag=f"q{m}", name=f"qT{m}")
            nc.scalar.copy(q_t[:], psum[:])
            qT.append(q_t)

        proj_T(wq, l, hq, DT8, QKV_DT, ev_q)

        # ---- context assembly from AG output ----
        kctx = []
        for dt in range(DT8):
            kc = kcp.tile([128, T], ATT_DT, tag=f"kc{dt}", name=f"kctx{dt}")
            nc.sync.dma_start(kc[:, 0:NTOK], kv_out[dt * 128:(dt + 1) * 128, :])
            nc.sync.dma_start(kc[:, NTOK:T],
                              kv_out[4 * NTOK + dt * 128:4 * NTOK + (dt + 1) * 128, :])
            kctx.append(kc)
        vaug = []
        for ct in range(8):
            va = vcp.tile([128, H * (HD + 1)], ATT_DT, tag=f"vc{ct}", name=f"vaug{ct}")
            if ct < 4:
                src = kv_out[2 * NTOK + ct * 256:2 * NTOK + (ct + 1) * 256, :]
            else:
                src = kv_out[6 * NTOK + (ct - 4) * 256:6 * NTOK + (ct - 3) * 256, :]
            nc.sync.dma_start(
                va[:].rearrange("p (h e) -> p h e", h=H)[:, :, 0:HD],
                src.rearrange("(p r) n -> p (r n)", r=2)
                   .rearrange("p (h e) -> p h e", h=H))
            nc.vector.memset(
                va[:].rearrange("p (h e) -> p h e", h=H)[:, :, HD:HD + 1], 1.0)
            vaug.append(va)

        # ---- attention ----
        rs = rsp.tile([1, H * NTOK], F32R, tag="rs")
        oT = [op.tile([128, NTOK], WO_DT, tag=f"o{dt}", name=f"oT{dt}") for dt in range(DT8)]
        for h in range(H):
            hoff = (h % 2) * 64
            ops = po.tile([HD + 1, NTOK], F32, tag="po")
            for ct in range(8):
                c0 = 0 if ct < 4 else (ct - 4) * 128
                sps = pg.tile([128, NTOK], F32, tag="pg")
                nc.tensor.matmul(sps[:, c0:NTOK],
                                 kctx[h // 2][hoff:hoff + 64,
                                              ct * 128:(ct + 1) * 128],
                                 qT[h // 2][hoff:hoff + 64, c0:NTOK],
                                 start=True, stop=True)
                wei = weip.tile([128, NTOK], ATT_DT, tag="wei")
                nc.scalar.activation(wei[:, c0:NTOK], sps[:, c0:NTOK],
                                     AF.Exp, scale=SCALE)
                nc.vector.tensor_mul(wei[:, c0:NTOK], wei[:, c0:NTOK],
                                     maskt[:, ct * NTOK + c0:(ct + 1) * NTOK])
                nc.tensor.matmul(ops[:, c0:NTOK],
                                 vaug[ct][:, h * 65:h * 65 + 65],
                                 wei[:, c0:NTOK],
                                 start=(ct == 0), stop=(ct == 7))
            nc.scalar.copy(oT[h // 2][hoff:hoff + 64, :], ops[0:HD, :])
            nc.scalar.copy(rs[0:1, h * NTOK:(h + 1) * NTOK], ops[HD:HD + 1, :])

        for dt in range(DT8):
            bps = ps.tile([128, NTOK], F32, tag="st")
            nc.tensor.matmul(bps[:], e_lo_r[:],
                             rs[0:1, (2 * dt) * NTOK:(2 * dt + 1) * NTOK],
                             start=True, stop=False)
            nc.tensor.matmul(bps[:], e_hi_r[:],
                             rs[0:1, (2 * dt + 1) * NTOK:(2 * dt + 2) * NTOK],
                             start=False, stop=True)
            rcp = scr.tile([128, NTOK], ATT_DT, tag="rcp")
            with nc.allow_low_precision(reason="softmax denom recip"):
                nc.vector.reciprocal(rcp[:], bps[:])
            nc.vector.tensor_mul(oT[dt][:], oT[dt][:], rcp[:])

        # ---- attn out proj + residual ----
        def ev_wo(m, psum):
            nc.vector.scalar_tensor_tensor(
                xT[m][:], psum[:], ppt[:, PP_BO + l * 8 + m:PP_BO + l * 8 + m + 1],
                xT[m][:], op0=ALU.add, op1=ALU.add)

        proj_T(wo, l, oT, DT8, WO_DT, ev_wo)

        # ---- LN2 + MLP ----
        h2 = layer_norm(PP_LN2G + l * 8, PP_LN2B + l * 8, W1_DT, hbp)

        uT = []

        def ev_u(m, psum):
            u_t = up.tile([128, NTOK], W2_DT, tag=f"u{m}", name=f"uT{m}")
            nc.scalar.activation(u_t[:], psum[:], AF.Relu,
                                 bias=ppt[:, PP_B1 + l * 32 + m:PP_B1 + l * 32 + m + 1])
            uT.append(u_t)

        proj_T(w1, l, h2, HT32, W1_DT, ev_u)

        def ev_w2(m, psum):
            nc.vector.scalar_tensor_tensor(
                xT[m][:], psum[:], ppt[:, PP_B2 + l * 8 + m:PP_B2 + l * 8 + m + 1],
                xT[m][:], op0=ALU.add, op1=ALU.add)

        proj_T(w2, l, uT, DT8, W2_DT, ev_w2)

    # ---- final LN + lm head + loss ----
    xb = layer_norm(PP_LNFG, PP_LNFB, LM_DT, hbp)

    sums = [sep.tile([128, NVC + 1], F32, tag=f"se{ti}", name=f"sums{ti}") for ti in range(TI4)]
    for vc in range(NVC):
        nv = min(512, VOCAB - vc * 512)
        wts = []
        for dt in range(DT8):
            wt = qp.tile([128, NTOK], LM_DT, tag=f"q{dt}", name=f"wlmt{dt}")
            nc.sync.dma_start(wt[:, :nv],
                              wlm[dt * 128:(dt + 1) * 128, vc * 512:vc * 512 + nv])
            wts.append(wt)
        for ti in range(TI4):
            pps = pg.tile([128, NTOK], F32, tag="pg")
            if has_blm:
                nc.tensor.matmul(pps[:, :nv], ones1_b[:],
                                 blmt[:, vc * 512:vc * 512 + nv],
                                 start=True, stop=False)
            for dt in range(DT8):
                nc.tensor.matmul(pps[:, :nv],
                                 xb[dt][:, ti * 128:(ti + 1) * 128],
                                 wts[dt][:, :nv],
                                 start=(dt == 0 and not has_blm),
                                 stop=(dt == DT8 - 1))
            lg = lgp.tile([128, NTOK], F32, tag="lg")
            nc.scalar.copy(lg[:, :nv], pps[:, :nv])
            nc.sync.dma_start(
                logits_o[ti * 128:(ti + 1) * 128, vc * 512:vc * 512 + nv],
                lg[:, :nv])
            esc = scr.tile([128, NTOK], F32R, tag="scr")
            nc.scalar.activation(esc[:, :nv], pps[:, :nv], AF.Exp,
                                 accum_out=sums[ti][:, vc:vc + 1])

    lps = ps.tile([128, NTOK], F32, tag="st")
    for ti in range(TI4):
        s_t = smp.tile([128, 1], F32, tag="s1")
        nc.vector.reduce_sum(s_t[:], sums[ti][:, 0:NVC], axis=mybir.AxisListType.X)
        lse = smp.tile([128, 1], F32, tag="s1")
        nc.scalar.activation(lse[:], s_t[:], AF.Ln)

        tps = ps.tile([128, NTOK], F32, tag="st")
        if has_blm:
            nc.tensor.matmul(tps[:, 0:128], ones1_b[:],
                             blmtgt[:, ti * 128:(ti + 1) * 128],
                             start=True, stop=False)
        for dt in range(DT8):
            wgt_t = wgp.tile([128, 128], LM_DT, tag="wg", name=f"wgt{ti}_{dt}")
            nc.sync.dma_start(wgt_t[:],
                              wg[dt * 128:(dt + 1) * 128, ti * 128:(ti + 1) * 128])
            nc.tensor.matmul(tps[:, 0:128],
                             xb[dt][:, ti * 128:(ti + 1) * 128],
                             wgt_t[:],
                             start=(dt == 0 and not has_blm),
                             stop=(dt == DT8 - 1))
        dsc = scr.tile([128, NTOK], F32, tag="scr")
        nc.vector.tensor_mul(dsc[:, 0:128], tps[:, 0:128], idt[:])
        tg = smp.tile([128, 1], F32, tag="s1")
        nc.vector.reduce_sum(tg[:], dsc[:, 0:128], axis=mybir.AxisListType.X)
        lv = smp.tile([128, 1], F32, tag="s1")
        nc.vector.tensor_sub(lv[:], lse[:], tg[:])
        nc.tensor.matmul(lps[0:1, 0:1], lv[:], ones_f[:],
                         start=(ti == 0), stop=(ti == TI4 - 1))
    lsb = smp.tile([1, 1], F32, tag="s1")
    nc.scalar.copy(lsb[:], lps[0:1, 0:1])
    nc.sync.dma_start(loss_o[:], lsb[:])

    for p in reversed(ctx_mgr):
        p.__exit__(None, None, None)


_CACHE = {}


def _get_nc(has_blm):
    key = ("nc", has_blm)
    if key not in _CACHE:
        _CACHE[key] = _build(has_blm)
    return _CACHE[key]


def _pack_pp(vec, lrows=True):
    """[L, n] (or [n]) f32 -> [128, n*L/128] with col (l, c) = vec[l, c*128:...]."""
    v = np.asarray(vec, np.float32)
    if v.ndim == 1:
        v = v[None]
    Lx, n = v.shape
    return v.reshape(Lx, n // 128, 128).transpose(2, 0, 1).reshape(128, -1)


def prepare_in_maps(idx, targets, params):
    idx = np.asarray(idx)
    targets = np.asarray(targets)
    p = {k: np.asarray(v) for k, v in params.items()
         if not isinstance(v, dict)}
    lay = {k: np.asarray(v) for k, v in params["layers"].items()}

    qkv_np = _np_dt(QKV_DT)
    att_np = _np_dt(ATT_DT)
    wo_np = _np_dt(WO_DT)
    w1_np = _np_dt(W1_DT)
    w2_np = _np_dt(W2_DT)
    lm_np = _np_dt(LM_DT)

    b_lm = p["b_lm"].astype(np.float32)
    has_blm = bool(np.any(b_lm != 0))

    shared = {
        "wq": np.ascontiguousarray(
            lay["wq"].transpose(0, 2, 1, 3).reshape(L, D, D).astype(qkv_np)),
        "wk": np.ascontiguousarray(
            lay["wk"].transpose(0, 2, 1, 3).reshape(L, D, D).astype(qkv_np)),
        "wv": np.ascontiguousarray(
            lay["wv"].transpose(0, 2, 1, 3).reshape(L, D, D).astype(qkv_np)),
        "wo": np.ascontiguousarray(lay["wo"].astype(wo_np)),
        "w1": np.ascontiguousarray(lay["w1"].astype(w1_np)),
        "w2": np.ascontiguousarray(lay["w2"].astype(w2_np)),
        "wlm": np.ascontiguousarray(p["w_lm"].astype(lm_np)),
        "blm": np.ascontiguousarray(b_lm[None, :].astype(lm_np)),
        "idm": np.eye(128, dtype=np.float32),
    }
    ppm = np.zeros((128, PP_COLS), np.float32)
    ppm[:, PP_LN1G:PP_LN1G + 64] = _pack_pp(lay["ln1_g"])
    ppm[:, PP_LN1B:PP_LN1B + 64] = _pack_pp(lay["ln1_b"])
    ppm[:, PP_LN2G:PP_LN2G + 64] = _pack_pp(lay["ln2_g"])
    ppm[:, PP_LN2B:PP_LN2B + 64] = _pack_pp(lay["ln2_b"])
    ppm[:, PP_BO:PP_BO + 64] = _pack_pp(lay["bo"])
    ppm[:, PP_B2:PP_B2 + 64] = _pack_pp(lay["b2"])
    ppm[:, PP_B1:PP_B1 + 256] = _pack_pp(lay["b1"])
    ppm[:, PP_LNFG:PP_LNFG + 8] = _pack_pp(p["lnf_g"])
    ppm[:, PP_LNFB:PP_LNFB + 8] = _pack_pp(p["lnf_b"])
    shared["pp"] = ppm

    tok_emb = p["tok_emb"].astype(np.float32)
    pos_emb = p["pos_emb"].astype(np.float32)
    w_lm_f = p["w_lm"].astype(np.float32)

    in_maps = []
    for c in range(NCORES):
        bidx, half = c // 2, c % 2
        sl = slice(half * NTOK, (half + 1) * NTOK)
        toks = idx[bidx, sl]
        tgts = targets[bidx, sl]
        m = dict(shared)
        m["x0tok"] = np.ascontiguousarray(tok_emb[toks].T)
        m["x0pos"] = np.ascontiguousarray(pos_emb[sl].T)
        qpos = half * NTOK + np.arange(NTOK)
        mask = np.zeros((128, 8 * NTOK), np.float32)
        for ct in range(8):
            kpos = ct * 128 + np.arange(128)
            mask[:, ct * NTOK:(ct + 1) * NTOK] = (
                kpos[:, None] <= qpos[None, :]).astype(np.float32)
        m["maskd"] = mask.astype(att_np)
        m["wg"] = np.ascontiguousarray(w_lm_f[:, tgts].astype(lm_np))
        m["blmtg"] = np.ascontiguousarray(b_lm[tgts][None, :].astype(lm_np))
        in_maps.append(m)
    return in_maps, has_blm


def assemble_outputs(results):
    logits = np.empty((B, T, VOCAB), np.float32)
    loss_sum = 0.0
    for c in range(NCORES):
        bidx, half = c // 2, c % 2
        logits[bidx, half * NTOK:(half + 1) * NTOK, :] = results[c]["logits_o"]
        loss_sum += float(results[c]["loss_o"][0, 0])
    loss = np.float32(loss_sum / (B * T))
    return logits, loss


def kernel(idx, targets, params):
    in_maps, has_blm = prepare_in_maps(idx, targets, params)
    nc = _get_nc(has_blm)
    res = run_bass_kernel_spmd(nc, in_maps, core_ids=list(range(NCORES)),
                               **_RUN_KW)
    _CACHE["last_results"] = res
    return assemble_outputs(res.results)




# knobs for test.py: extra kwargs for run_bass_kernel_spmd (e.g. trace=True),
# and _CACHE["last_results"] holds the BassKernelResults of the last run.
_RUN_KW = {}
